# revision 1
# baseline (speedup 1.0000x reference)
"""Trainium2 Bass kernel for nn_ItemEncoder.

Computation:
    h_type = emb[item_type]                      # [bs, na, ni, 32]
    h = concat([h_type, item], -1)               # [bs, na, ni, 43]
    z = h @ W + b                                # [bs, na, ni, 128]
    out = max_{ni} relu(z)                       # [bs, na, 128]

Device strategy (pure data parallel over bs, 4 batches/core):
    Fold the embedding gather + bias into the matmul:
        T = emb @ W[:32] + b                     # (18, 128) tiny table
        z_tok = [x_tok ; onehot(t_tok)] @ [W2 ; T]   (K = 11 + 18 = 29)
    Host packs rhs [29, ntok] (features + one-hot), device runs K=29
    matmuls into PSUM and reduces max over ni=128 token groups on DVE
    (relu commutes with max, applied once at the end on the [128, 512]
    result).  Output is [h=128, group=512] per core; host transposes.
"""

import sys

sys.path.insert(0, "/opt/trn_rl_repo")

import ml_dtypes
import numpy as np

import concourse.bass as bass
import concourse.tile as tile
from concourse import bacc, mybir
from concourse import bass_utils

BS, NA, NI, F, H = 32, 128, 128, 11, 128
NTYPE, KEMB = 18, 32
NCORES = 8
BPC = BS // NCORES          # batches per core = 4
G = BPC * NA                # (b, na) groups per core = 512
TOK = G * NI                # tokens per core = 65536
K = F + NTYPE               # contraction dim = 29
KP = 32                     # K padded to 32 partitions (zeros) for DMA alignment
CHUNK = 2048                # tokens per chunk (4 psum banks)
NCHUNK = TOK // CHUNK       # 32
F32 = mybir.dt.float32
BF16 = mybir.dt.bfloat16

_cache = {}


def _build_program(repeat=1):
    key = ("nc", repeat)
    if key in _cache:
        return _cache[key]

    nc = bacc.Bacc(
        "TRN2",
        target_bir_lowering=False,
        debug=False,
        enable_asserts=False,
        num_devices=NCORES,
    )

    rhs_d = nc.dram_tensor("rhs", [NCHUNK, KP, CHUNK], BF16, kind="ExternalInput").ap()
    lhsT_d = nc.dram_tensor("lhsT", [KP, H], BF16, kind="ExternalInput").ap()
    out_d = nc.dram_tensor("out", [H, G], F32, kind="ExternalOutput").ap()

    with tile.TileContext(nc) as tc:
        with (
            tc.tile_pool(name="const", bufs=1) as const_pool,
            tc.tile_pool(name="rhs", bufs=4) as rhs_pool,
            tc.tile_pool(name="ps", bufs=2, space=bass.MemorySpace.PSUM) as ps_pool,
            tc.tile_pool(name="res", bufs=2) as res_pool,
        ):
            lt = const_pool.tile([KP, H], BF16)
            nc.sync.dma_start(lt[:], lhsT_d[:])

            def body():
                ob = res_pool.tile([H, G], F32)
                orelu = res_pool.tile([H, G], F32)

                for j in range(NCHUNK):
                    r = rhs_pool.tile([KP, CHUNK], BF16)
                    nc.sync.dma_start(r[:], rhs_d[j])

                    p = ps_pool.tile([H, CHUNK], F32)
                    for k in range(CHUNK // 512):
                        nc.tensor.matmul(
                            p[:, k * 512:(k + 1) * 512],
                            lt[:],
                            r[:, k * 512:(k + 1) * 512],
                        )

                    # max over ni=128 within each (b, na) group
                    gpc = CHUNK // NI  # groups per chunk = 16
                    nc.vector.reduce_max(
                        ob[:, j * gpc:(j + 1) * gpc],
                        p[:].rearrange("p (g i) -> p g i", i=NI),
                        axis=mybir.AxisListType.X,
                    )

                nc.scalar.activation(
                    orelu[:], ob[:], mybir.ActivationFunctionType.Relu
                )
                nc.sync.dma_start(out_d[:], orelu[:])

            if repeat == 1:
                body()
            else:
                with tc.For_i(0, repeat, 1):
                    body()

    nc.compile()
    _cache[key] = nc
    return nc


def _pack_inputs(item_type, item, emb, W, b):
    T_tab = (emb.astype(np.float32) @ W[:KEMB].astype(np.float32)
             + b.astype(np.float32))                       # (18, 128)
    lhsT = np.concatenate(
        [W[KEMB:].astype(np.float32), T_tab], axis=0
    ).astype(ml_dtypes.bfloat16)                           # (29, 128)
    lhsT = np.concatenate(
        [lhsT, np.zeros((KP - K, H), dtype=ml_dtypes.bfloat16)], axis=0
    )                                                      # (32, 128)
    eye = np.eye(NTYPE, dtype=ml_dtypes.bfloat16)

    in_maps = []
    for c in range(NCORES):
        x = item[c * BPC:(c + 1) * BPC].astype(np.float32).reshape(TOK, F)
        t = np.asarray(item_type[c * BPC:(c + 1) * BPC]).reshape(TOK)
        rhs = np.zeros((KP, TOK), dtype=ml_dtypes.bfloat16)
        rhs[:F] = x.T.astype(ml_dtypes.bfloat16)
        rhs[F:K] = eye[t].T                                 # one-hot rows
        rhs = np.ascontiguousarray(
            rhs.reshape(KP, NCHUNK, CHUNK).transpose(1, 0, 2)
        )                                                   # (32, 32, 2048)
        in_maps.append({"rhs": rhs, "lhsT": lhsT})
    return in_maps


def _run(in_maps, trace=False, repeat=1):
    nc = _build_program(repeat)
    return bass_utils.run_bass_kernel_spmd(
        nc, in_maps, core_ids=list(range(NCORES)), trace=trace
    )


def kernel(item_type, item, emb, W, b):
    in_maps = _pack_inputs(item_type, item, emb, W, b)
    res = _run(in_maps, trace=False)
    out = np.empty((BS, NA, H), dtype=np.float32)
    for c in range(NCORES):
        o = res.results[c]["out"]                           # (128, 512) [h, g]
        out[c * BPC:(c + 1) * BPC] = o.T.reshape(BPC, NA, H)
    return out



# revision 2
# speedup vs baseline: 1.7548x; 1.7548x over previous
"""Trainium2 Bass kernel for nn_ItemEncoder.

Computation:
    h_type = emb[item_type]                      # [bs, na, ni, 32]
    h = concat([h_type, item], -1)               # [bs, na, ni, 43]
    z = h @ W + b                                # [bs, na, ni, 128]
    out = max_{ni} relu(z)                       # [bs, na, 128]

Device strategy (pure data parallel over bs, 4 batches/core):
    Fold the embedding gather + bias into the matmul:
        T = emb @ W[:32] + b                     # (18, 128) tiny table
        z_tok = [x_tok ; onehot(t_tok)] @ [W2 ; T]   (K = 11 + 18 = 29)
    rhs is packed host-side as [128, NCHUNK, 512] bf16: each chunk of
    2048 tokens splits into 4 streams of 512, one per 32-partition band,
    so the 4 matmuls per chunk use 4-way PE row tiling (tile_position)
    and the LoadStationary of one tile overlaps the streaming of another.

    PSUM egress (the real bottleneck: 65536 elem/partition/core must
    leave PSUM through DVE or ScalarE) is handled by ScalarE Relu-evac
    to SBUF bf16 (~2 elem/cycle), split into two 8-group ops per chunk
    so PSUM banks recycle at half-chunk granularity (keeps the PE fed).
    DVE runs only pairwise-max trees on the bf16 data (2 results/cycle),
    batched per 4-chunk block and interleaved across block pairs so
    consecutive DVE ops are independent (dodges the DVE pipeline-drain
    penalty, which only hurts RAW-dependent chains).
    Final Relu over the assembled [128, 512] result, then one out DMA.
"""

import sys

sys.path.insert(0, "/opt/trn_rl_repo")

import ml_dtypes
import numpy as np

import concourse.bass as bass
import concourse.tile as tile
from concourse import bacc, mybir
from concourse import bass_utils

BS, NA, NI, F, H = 32, 128, 128, 11, 128
NTYPE, KEMB = 18, 32
NCORES = 8
BPC = BS // NCORES          # batches per core = 4
G = BPC * NA                # (b, na) groups per core = 512
TOK = G * NI                # tokens per core = 65536
K = F + NTYPE               # contraction dim = 29
KP = 32                     # padded to 32 (zeros)
CHUNK = 2048                # tokens per chunk
NCHUNK = TOK // CHUNK       # 32
GPC = CHUNK // NI           # groups per chunk = 16
NB = 4                      # chunks per block (DMA + tree granularity)
NBLK = NCHUNK // NB         # 8
F32 = mybir.dt.float32
BF16 = mybir.dt.bfloat16
MAX = mybir.AluOpType.max

_cache = {}


def _build_program(repeat=1):
    key = ("v3", repeat)
    if key in _cache:
        return _cache[key]

    nc = bacc.Bacc(
        "TRN2",
        target_bir_lowering=False,
        debug=False,
        enable_asserts=False,
        num_devices=NCORES,
    )

    rhs_d = nc.dram_tensor("rhs", [128, NCHUNK, 512], BF16, kind="ExternalInput").ap()
    lhsT_d = nc.dram_tensor("lhsT", [128, H], BF16, kind="ExternalInput").ap()
    out_d = nc.dram_tensor("out", [H, G], F32, kind="ExternalOutput").ap()

    with tile.TileContext(nc) as tc, \
         tc.tile_pool(name="const", bufs=1) as const_pool, \
         tc.tile_pool(name="rhs", bufs=3) as rhs_pool, \
         tc.tile_pool(name="ps", bufs=2, space=bass.MemorySpace.PSUM) as ps_pool, \
         tc.tile_pool(name="evac", bufs=3) as evac_pool, \
         tc.tile_pool(name="tmp", bufs=2) as tmp_pool, \
         tc.tile_pool(name="res", bufs=1) as res_pool:

        lt = const_pool.tile([128, H], BF16)
        nc.sync.dma_start(lt[:], lhsT_d[:])

        def emit_block_compute(n):
            """DMA + matmuls + split ScE evac for block n; returns evac tile."""
            r4 = rhs_pool.tile([128, NB, 512], BF16, name="r4")
            nc.sync.dma_start(r4[:], rhs_d[:, n * NB:(n + 1) * NB, :])
            eb = evac_pool.tile([H, NB, GPC, NI], BF16, name="eb")
            for jj in range(NB):
                p = ps_pool.tile([H, CHUNK], F32, name="p")
                p3 = p[:].rearrange("p (g i) -> p g i", i=NI)
                for k in range(4):
                    nc.tensor.matmul(
                        p[:, k * 512:(k + 1) * 512],
                        lt[k * 32:(k + 1) * 32, :],
                        r4[k * 32:(k + 1) * 32, jj, :],
                        tile_position=(k * 32, 0),
                    )
                # evac split in two: PSUM banks recycle at half-chunk grain
                for h in range(2):
                    nc.scalar.activation(
                        eb[:, jj, h * 8:(h + 1) * 8, :],
                        p3[:, h * 8:(h + 1) * 8, :],
                        mybir.ActivationFunctionType.Relu,
                    )
            return eb

        def tree_levels(n, eb, ob4):
            """(dst, in0, in1) for the 7 pairwise-max levels of block n."""
            t64 = tmp_pool.tile([H, NB, GPC, 64], BF16, name="t64")
            t32 = tmp_pool.tile([H, NB, GPC, 32], BF16, name="t32")
            levels = []
            src, n_items = eb, NI
            tmps = [t64, t32, t64, t32, t64, t32]
            for li in range(7):
                half = n_items // 2
                if half == 1:
                    dst = ob4[:, n * NB:(n + 1) * NB, :, :]
                else:
                    dst = tmps[li][:, :, :, 0:half]
                levels.append(
                    (dst, src[:, :, :, 0:half], src[:, :, :, half:n_items])
                )
                if li < 6:
                    src = tmps[li]
                n_items = half
            return levels

        def body():
            ob = res_pool.tile([H, G], F32, name="ob")
            orelu = res_pool.tile([H, G], F32, name="orelu")
            ob4 = ob[:].rearrange("p (c g i) -> p c g i", g=GPC, i=1)

            for m in range(NBLK // 2):
                na_, nb_ = 2 * m, 2 * m + 1
                eba = emit_block_compute(na_)
                ebb = emit_block_compute(nb_)
                la = tree_levels(na_, eba, ob4)
                lb = tree_levels(nb_, ebb, ob4)
                for (da, a0, a1), (db, b0, b1) in zip(la, lb):
                    nc.vector.scalar_tensor_tensor(da, a0, 0.0, a1, op0=MAX, op1=MAX)
                    nc.vector.scalar_tensor_tensor(db, b0, 0.0, b1, op0=MAX, op1=MAX)

            nc.scalar.activation(
                orelu[:], ob[:], mybir.ActivationFunctionType.Relu
            )
            nc.sync.dma_start(out_d[:], orelu[:])

        if repeat == 1:
            body()
        else:
            with tc.For_i(0, repeat, 1):
                body()

    nc.compile()
    _cache[key] = nc
    return nc


def _pack_inputs(item_type, item, emb, W, b):
    T_tab = (emb.astype(np.float32) @ W[:KEMB].astype(np.float32)
             + b.astype(np.float32))                       # (18, 128)
    lhsT = np.concatenate(
        [W[KEMB:].astype(np.float32), T_tab], axis=0
    ).astype(ml_dtypes.bfloat16)                           # (29, 128)
    lhsT = np.concatenate(
        [lhsT, np.zeros((KP - K, H), dtype=ml_dtypes.bfloat16)], axis=0
    )                                                      # (32, 128)
    lhsT4 = np.ascontiguousarray(np.tile(lhsT, (4, 1)))    # (128, 128)
    eye = np.eye(NTYPE, dtype=ml_dtypes.bfloat16)

    in_maps = []
    for c in range(NCORES):
        x = item[c * BPC:(c + 1) * BPC].astype(np.float32).reshape(TOK, F)
        t = np.asarray(item_type[c * BPC:(c + 1) * BPC]).reshape(TOK)
        rhs = np.zeros((KP, TOK), dtype=ml_dtypes.bfloat16)
        rhs[:F] = x.T.astype(ml_dtypes.bfloat16)
        rhs[F:K] = eye[t].T                                 # one-hot rows
        # [KP, TOK] -> [KP, NCHUNK, 4, 512]; partition p = 32*stream + k
        rhs = np.ascontiguousarray(
            rhs.reshape(KP, NCHUNK, 4, 512).transpose(2, 0, 1, 3)
        ).reshape(128, NCHUNK, 512)
        in_maps.append({"rhs": rhs, "lhsT": lhsT4})
    return in_maps


def _run(in_maps, trace=False, repeat=1):
    nc = _build_program(repeat)
    return bass_utils.run_bass_kernel_spmd(
        nc, in_maps, core_ids=list(range(NCORES)), trace=trace
    )


def kernel(item_type, item, emb, W, b):
    in_maps = _pack_inputs(item_type, item, emb, W, b)
    res = _run(in_maps, trace=False)
    out = np.empty((BS, NA, H), dtype=np.float32)
    for c in range(NCORES):
        o = res.results[c]["out"]                           # (128, 512) [h, g]
        out[c * BPC:(c + 1) * BPC] = o.T.reshape(BPC, NA, H)
    return out
